# revision 1
# baseline (speedup 1.0000x reference)
"""CGCNN TRN2 kernel: 8-core SPMD edge pipeline + single-core pooling/classifier.

v2: layer-1 Z streamed from host (no gathers), layers 2/3 gather only x[src]
(x[dst] expanded on-chip via transposed one-hot matmuls), merge accumulates
in PSUM, batched Sel build, native Softplus.
"""
import numpy as np

G = 16
SLOTS = 2
USE_SOFTPLUS = False  # Softplus absent from this target's activation tables


def preprocess(x, edge_index, edge_attr, batch, params, n_dev=8, ncol=512):
    N = x.shape[0]
    npd = N // n_dev
    ntiles = (npd + 127) // 128
    spsb = SLOTS * ncol          # slots per superblock (1024)
    src = np.asarray(edge_index[0], dtype=np.int64)
    dst = np.asarray(edge_index[1], dtype=np.int64)
    attr = np.asarray(edge_attr, dtype=np.float32)
    xf = np.asarray(x, np.float32)

    # per (dev, tile) slot counts
    devdata = []
    D8 = 0
    for d in range(n_dev):
        lo = d * npd
        sel = np.where((dst >= lo) & (dst < lo + npd))[0]
        order = np.argsort(dst[sel], kind="stable")
        eid = sel[order]
        ed = (dst[eid] - lo).astype(np.int64)
        cnt = np.bincount(ed, minlength=npd)
        scnt = (cnt + 7) // 8                       # 8-slots per node
        tile_of_node = np.arange(npd) // 128
        tslots = np.bincount(tile_of_node, weights=scnt, minlength=ntiles).astype(np.int64)
        D8 = max(D8, int(tslots.max()))
        devdata.append((lo, eid, ed, cnt, scnt))
    D8 = int(D8)

    nslots = ntiles * D8
    nsb = (nslots + spsb - 1) // spsb
    out = []
    for d in range(n_dev):
        lo, eid, ed, cnt, scnt = devdata[d]
        # slot index of each node's first slot
        node_slot0 = np.zeros(npd, np.int64)
        for T in range(ntiles):
            nlo = T * 128
            nhi = min(nlo + 128, npd)
            c = np.concatenate(([0], np.cumsum(scnt[nlo:nhi])[:-1]))
            node_slot0[nlo:nhi] = T * D8 + c
        estart = np.concatenate(([0], np.cumsum(cnt)[:-1]))  # first sorted-edge per node
        rank = np.arange(len(ed)) - estart[ed]
        slot = node_slot0[ed] + rank // 8
        # slot -> (sb, n, s); edge -> (sb, n, g)
        si = slot
        sb_e = si // spsb
        rem = si % spsb
        n_e = rem // SLOTS
        s_e = rem % SLOTS
        g_e = 8 * s_e + (rank % 8)
        t_e = n_e // 128
        p_e = n_e % 128

        gidx = np.zeros((nsb, 128, 64), np.int32)
        gidx[sb_e, p_e, t_e * 16 + g_e] = src[eid]
        attr_sw = np.zeros((nsb, 80, ncol), np.float32)
        for c in range(4):
            attr_sw[sb_e, g_e * 5 + c, n_e] = attr[eid, c]
        # rho row occupies 5g+4; 1.0 for pads, 0.0 for real edges
        pad_rho = np.ones((nsb, G, ncol), np.float32)
        pad_rho[sb_e, g_e, n_e] = 0.0
        for gg in range(G):
            attr_sw[:, gg * 5 + 4, :] = pad_rho[:, gg, :]

        # per-slot node (relative): -1 for pad slots
        nd_of = np.full(nsb * spsb, -1, np.int64)
        for T in range(ntiles):
            nlo = T * 128
            nhi = min(nlo + 128, npd)
            k = int(scnt[nlo:nhi].sum())
            nd_of[T * D8:T * D8 + k] = np.repeat(np.arange(nlo, nhi), scnt[nlo:nhi])
        sn = nd_of.reshape(nsb, ncol, SLOTS)   # [sb, n, s]

        # mrel: [nsb, 128, 16] int8: col 2*(2c+s)+part
        mrelv = np.full((nsb, 128, 16), -1, np.int8)
        for c in range(4):
            for s in range(SLOTS):
                ncols = np.arange(128 * c, 128 * (c + 1))
                v = sn[:, ncols, s]                # [nsb, 128]
                iota_slot = (np.arange(nsb) * spsb)[:, None] + ncols[None, :] * SLOTS + s
                Tt = iota_slot // D8
                rel = v - 128 * Tt
                rel[v < 0] = -1
                assert ((rel >= -1) & (rel < 128)).all()
                T0 = Tt[:, 0:1]
                for part in range(2):
                    rp = np.where((Tt == T0 + part) & (rel >= 0), rel, -1)
                    mrelv[:, :, 2 * (2 * c + s) + part] = rp.astype(np.int8)

        # layer-1 Z stream [nsb, 54, ncol] (host pre-gathered x)
        z1 = np.zeros((nsb, 54, ncol), np.float32)
        xv = xf[gidx[:, :, :]]                     # [nsb, 128, 64, 3]
        # z1[b, 3g+f, 128t+p] = xv[b, p, 16t+g, f]
        xv = xv.reshape(nsb, 128, 4, 16, 3)
        z1[:, 0:48, :] = xv.transpose(0, 3, 4, 2, 1).reshape(nsb, 48, ncol)
        snv = np.maximum(sn, 0)
        xd = np.where(sn[..., None] >= 0, xf[lo + snv], 0.0)  # [nsb, ncol, s, 3]
        z1[:, 48:54, :] = xd.transpose(0, 2, 3, 1).reshape(nsb, 6, ncol)

        xsl = np.zeros((ntiles * 128, 3), np.float32)
        xsl[:npd] = xf[lo:lo + npd]
        out.append(dict(gidx=gidx, attr_sw=attr_sw, mrel=mrelv, xsl=xsl,
                        z1=z1, lo=lo))

    # merge schedule: per (sb, c, s): list of (T, k) with k = Sel column block
    msched = []
    for b in range(nsb):
        ent_b = []
        for c in range(4):
            for s in range(SLOTS):
                io0 = b * spsb + (128 * c) * 2 + s
                iolast = io0 + 127 * 2
                T0 = io0 // D8
                T1 = iolast // D8
                col0 = 2 * (2 * c + s)
                if T0 == T1:
                    parts = [(T0, col0)]
                else:
                    assert T1 == T0 + 1, "D8 too small"
                    parts = [(T0, col0), (T0 + 1, col0 + 1)]
                ent_b.append((c, s, parts))
        msched.append(ent_b)

    # start/stop bookkeeping for PSUM agg accumulation: order of (b, entry)
    touch = {}
    for b in range(nsb):
        for (c, s, parts) in msched[b]:
            for (T, k) in parts:
                if T < ntiles:
                    touch.setdefault(T, []).append((b, c, s, k))
    first_touch = {T: v[0] for T, v in touch.items()}
    last_touch = {T: v[-1] for T, v in touch.items()}
    assert len(touch) == ntiles

    WT = {}
    for l in (1, 2, 3):
        Wf, bf = params[f"Wf{l}"], params[f"bf{l}"]
        Ws_, bs = params[f"Ws{l}"], params[f"bs{l}"]
        Wxd = np.zeros((128, 112), np.float32)
        Wa = np.zeros((5 * G, 112), np.float32)
        for gg in range(G):
            ss = gg // 8
            for j in range(3):
                mg = 3 * gg + j
                mc = 64 + 3 * gg + j
                for f in range(3):
                    Wxd[3 * gg + f, mg] = Wf[j, 3 + f]
                    Wxd[3 * gg + f, mc] = Ws_[j, 3 + f]
                    Wxd[64 + 3 * ss + f, mg] = Wf[j, f]
                    Wxd[64 + 3 * ss + f, mc] = Ws_[j, f]
                for c in range(4):
                    Wa[5 * gg + c, mg] = Wf[j, 6 + c]
                    Wa[5 * gg + c, mc] = Ws_[j, 6 + c]
                Wa[5 * gg + 4, mc] = -30000.0
        bias_g = np.zeros((48, 1), np.float32)
        bias_c = np.zeros((48, 1), np.float32)
        for gg in range(G):
            for j in range(3):
                bias_g[3 * gg + j, 0] = bf[j]
                bias_c[3 * gg + j, 0] = bs[j]
        WT[l] = dict(Wxd=Wxd, Wa=Wa, bias_g=bias_g, bias_c=bias_c,
                     bias_cm30=bias_c - 30.0)

    S = np.zeros((48, 6), np.float32)
    for ss in range(SLOTS):
        for rr in range(8):
            for f in range(3):
                S[24 * ss + 3 * rr + f, 3 * ss + f] = 1.0
    iota16 = np.tile(np.arange(128, dtype=np.float32), (128, 16))

    shared = dict(WT=WT, S=S, iota16=iota16, msched=msched, nsb=nsb, ncol=ncol,
                  npd=npd, ntiles=ntiles, D8=D8, n_dev=n_dev,
                  first_touch=first_touch, last_touch=last_touch)
    return out, shared


import concourse.bass as bass
import concourse.bacc as bacc
import concourse.tile as tile
import concourse.mybir as mybir
from concourse.masks import make_identity

F32 = mybir.dt.float32
AF = mybir.ActivationFunctionType
OP = mybir.AluOpType


def build_spmd(shared, n_dev=8, N=100000, nreps=1):
    nsb, ncol, npd, ntiles = (shared["nsb"], shared["ncol"], shared["npd"],
                              shared["ntiles"])
    msched = shared["msched"]
    first_touch, last_touch = shared["first_touch"], shared["last_touch"]
    nc = bacc.Bacc("TRN2", target_bir_lowering=False, debug=False,
                   num_devices=n_dev)

    x_in = nc.dram_tensor("x", [N, 3], F32, kind="ExternalInput").ap()
    xsl_in = nc.dram_tensor("xsl", [ntiles * 128, 3], F32, kind="ExternalInput").ap()
    gidx = nc.dram_tensor("gidx", [nsb, 128, 64], mybir.dt.int32,
                          kind="ExternalInput").ap()
    attr_sw = nc.dram_tensor("attr_sw", [nsb, 80, ncol], F32,
                             kind="ExternalInput").ap()
    mrel = nc.dram_tensor("mrel", [nsb, 128, 16], mybir.dt.int8,
                          kind="ExternalInput").ap()
    z1_in = nc.dram_tensor("z1", [nsb, 54, ncol], F32, kind="ExternalInput").ap()
    wts = {}
    for l in (1, 2, 3):
        for nm, shp in (("Wxd", [128, 112]), ("Wa", [80, 112]),
                        ("bias_g", [48, 1]), ("bias_c", [48, 1]),
                        ("bias_cm30", [48, 1])):
            wts[(l, nm)] = nc.dram_tensor(f"{nm}{l}", shp, F32,
                                          kind="ExternalInput").ap()
    S_in = nc.dram_tensor("S", [48, 6], F32, kind="ExternalInput").ap()
    iota_in = nc.dram_tensor("iota16", [128, 16 * 128], F32,
                             kind="ExternalInput").ap()
    xouts = [nc.dram_tensor(f"xo{l}", [ntiles * 128, 3], F32,
                            kind="ExternalOutput").ap() for l in (1, 2, 3)]

    from contextlib import ExitStack
    with tile.TileContext(nc) as tc, ExitStack() as _es:
        cp = _es.enter_context(tc.tile_pool(name="const", bufs=1))
        ident = cp.tile([128, 128], F32)
        make_identity(nc, ident[:])
        iota_t = cp.tile([128, 16 * 128], F32)
        nc.sync.dma_start(out=iota_t[:], in_=iota_in[:])
        S_t = cp.tile([48, 6], F32)
        nc.sync.dma_start(out=S_t[:], in_=S_in[:])
        wt = {}
        for (l, nm), ap_ in wts.items():
            t = cp.tile(list(ap_.shape), F32, name=f"w{nm}{l}")
            nc.sync.dma_start(out=t[:], in_=ap_[:])
            wt[(l, nm)] = t
        x_sb = cp.tile([128, ntiles * 3], F32)

        dp = _es.enter_context(tc.tile_pool(name="dram", bufs=1, space="DRAM"))

        agg_sb = cp.tile([128, ntiles * 3], F32)
        sb = _es.enter_context(tc.tile_pool(name="work", bufs=8))
        ps = _es.enter_context(tc.tile_pool(name="psum", bufs=1, space="PSUM"))

        for rep in range(nreps):
          xfulls = [dp.tile([N, 3], F32, name=f"xfull{l}_r{rep}",
                            addr_space="Shared") for l in (1, 2)]
          xslo = [dp.tile([ntiles * 128, 3], F32, name=f"xslo{l}_r{rep}")
                  for l in (1, 2)]
          nc.sync.dma_start(
              out=x_sb[:].rearrange("p (T f) -> p T f", f=3),
              in_=xsl_in[:].rearrange("(T p) f -> p T f", p=128))
          for l in (1, 2, 3):
            xsrc = x_in if l == 1 else xfulls[l - 2][:]
            nc.vector.memset(agg_sb[:], 0.0)
            for b in range(nsb):
                mrt = sb.tile([128, 16], mybir.dt.int8, tag="mrt")
                nc.sync.dma_start(out=mrt[:], in_=mrel[b])
                mrf = sb.tile([128, 16], F32, tag="mrf")
                nc.vector.tensor_copy(out=mrf[:], in_=mrt[:])
                at = sb.tile([80, ncol], F32, tag="at", bufs=4)
                nc.sync.dma_start(out=at[:], in_=attr_sw[b])
                # batched Sel build: Sel16[p, 128k+j] = (mrf[p,k] == j)
                sel16 = sb.tile([128, 16 * 128], F32, tag="sel16", bufs=3)
                nc.vector.tensor_tensor(
                    out=sel16[:],
                    in0=mrf[:].rearrange("p (k o) -> p k o", o=1).to_broadcast(
                        [128, 16, 128]),
                    in1=iota_t[:].rearrange("p (k j) -> p k j", j=128),
                    op=OP.is_equal)
                Z = sb.tile([128, ncol], F32, tag="Z")
                if l == 1:
                    nc.sync.dma_start(out=Z[0:48, :], in_=z1_in[b, 0:48])
                    nc.sync.dma_start(out=Z[64:70, :], in_=z1_in[b, 48:54])
                else:
                    git = sb.tile([128, 64], mybir.dt.int32, tag="git",
                                  bufs=4)
                    nc.sync.dma_start(out=git[:], in_=gidx[b])
                    for t in range(4):
                        preT = sb.tile([128, 48], F32, tag="preT")
                        for g in range(G):
                            nc.gpsimd.indirect_dma_start(
                                out=preT[:, 3 * g:3 * g + 3], out_offset=None,
                                in_=xsrc,
                                in_offset=bass.IndirectOffsetOnAxis(
                                    ap=git[:, 16 * t + g:16 * t + g + 1],
                                    axis=0))
                        tp = ps.tile([48, 128], F32, tag="pss", bufs=3)
                        nc.tensor.transpose(out=tp[:], in_=preT[:],
                                            identity=ident[:])
                        nc.scalar.copy(out=Z[0:48, 128 * t:128 * (t + 1)],
                                       in_=tp[:])
                    # x[dst] expand, transposed: xdT_c[j, 3s+f] =
                    # x_tile[node(slot (128c+j, s))] via SelT matmuls, then
                    # transpose each chunk back into Z[64:70, 128c:...].
                    for c in range(4):
                        xdTs = sb.tile([128, 6], F32, tag="xdTs")
                        for (cc, s, parts) in msched[b]:
                            if cc != c:
                                continue
                            vparts = [(T, k) for (T, k) in parts if T < ntiles]
                            if not vparts:
                                vparts = [(ntiles - 1, parts[0][1])]
                            sels = []
                            for i, (T, k) in enumerate(vparts):
                                selT = ps.tile([128, 128], F32, tag="pss",
                                               bufs=3, name="selT")
                                nc.tensor.transpose(
                                    out=selT[:],
                                    in_=sel16[:, 128 * k:128 * (k + 1)],
                                    identity=ident[:])
                                selTs = sb.tile([128, 128], F32, tag="selTs")
                                if (c + s) % 2 == 0:
                                    nc.scalar.copy(out=selTs[:], in_=selT[:])
                                else:
                                    nc.vector.tensor_copy(out=selTs[:],
                                                          in_=selT[:])
                                sels.append((selTs, T))
                            xdT = ps.tile([128, 3], F32, tag="xdT", bufs=2)
                            for i, (selTs, T) in enumerate(sels):
                                nc.tensor.matmul(
                                    out=xdT[:],
                                    lhsT=selTs[:],
                                    rhs=x_sb[:, 3 * T:3 * T + 3],
                                    start=(i == 0),
                                    stop=(i == len(sels) - 1))
                            nc.scalar.copy(out=xdTs[:, 3 * s:3 * s + 3],
                                           in_=xdT[:])
                        xdp = ps.tile([6, 128], F32, tag="pss", bufs=3,
                                      name="xdp")
                        nc.tensor.transpose(out=xdp[:], in_=xdTs[:],
                                            identity=ident[:])
                        nc.scalar.copy(out=Z[64:70, 128 * c:128 * (c + 1)],
                                       in_=xdp[:])
                L = ps.tile([112, ncol], F32, tag="L", bufs=2)
                nc.tensor.matmul(out=L[:], lhsT=wt[(l, "Wxd")][0:48, :],
                                 rhs=Z[0:48, :], start=True, stop=False)
                nc.tensor.matmul(out=L[:], lhsT=wt[(l, "Wxd")][64:70, :],
                                 rhs=Z[64:70, :], start=False, stop=False)
                nc.tensor.matmul(out=L[:], lhsT=wt[(l, "Wa")][:], rhs=at[:],
                                 start=False, stop=True)
                gate = sb.tile([48, ncol], F32, tag="gate")
                nc.scalar.activation(out=gate[:], in_=L[0:48, :], func=AF.Sigmoid,
                                     bias=wt[(l, "bias_g")][:, 0:1])
                corev = sb.tile([48, ncol], F32, tag="corev")
                if USE_SOFTPLUS:
                    nc.scalar.activation(out=corev[:], in_=L[64:112, :],
                                         func=AF.Softplus,
                                         bias=wt[(l, "bias_c")][:, 0:1])
                else:
                    mcl = sb.tile([48, ncol], F32, tag="mcl")
                    nc.vector.tensor_scalar(
                        out=mcl[:], in0=L[64:112, :],
                        scalar1=wt[(l, "bias_c")][:, 0:1], scalar2=30.0,
                        op0=OP.add, op1=OP.min)
                    rl = sb.tile([48, ncol], F32, tag="rl")
                    nc.scalar.activation(out=rl[:], in_=L[64:112, :],
                                         func=AF.Relu,
                                         bias=wt[(l, "bias_cm30")][:, 0:1])
                    nc.scalar.activation(out=mcl[:], in_=mcl[:], func=AF.Exp)
                    nc.scalar.activation(out=corev[:], in_=mcl[:], func=AF.Ln,
                                         bias=1.0)
                    nc.vector.tensor_tensor(out=corev[:], in0=corev[:],
                                            in1=rl[:], op=OP.add)
                msg = sb.tile([48, ncol], F32, tag="msg")
                nc.vector.tensor_tensor(out=msg[:], in0=gate[:], in1=corev[:],
                                        op=OP.mult)
                p8 = ps.tile([6, ncol], F32, tag="pss", bufs=3, name="p8")
                nc.tensor.matmul(out=p8[:], lhsT=S_t[:], rhs=msg[:],
                                 start=True, stop=True)
                p8s = sb.tile([6, ncol], F32, tag="p8s")
                nc.scalar.copy(out=p8s[:], in_=p8[:])
                for c in range(4):
                    tp2 = ps.tile([128, 6], F32, tag="pss", bufs=3,
                                  name="tp2")
                    nc.tensor.transpose(out=tp2[:],
                                        in_=p8s[:, 128 * c:128 * (c + 1)],
                                        identity=ident[0:6, 0:6])
                    tps = sb.tile([128, 6], F32, tag="tps")
                    nc.scalar.copy(out=tps[:], in_=tp2[:])
                    for (cc, s, parts) in msched[b]:
                        if cc != c:
                            continue
                        for (T, k) in parts:
                            if T >= ntiles:
                                continue
                            selp = ps.tile([128, 3], F32, tag="pss", bufs=3,
                                           name="selp")
                            nc.tensor.matmul(
                                out=selp[:],
                                lhsT=sel16[:, 128 * k:128 * (k + 1)],
                                rhs=tps[:, 3 * s:3 * s + 3],
                                start=True, stop=True)
                            nc.vector.tensor_tensor(
                                out=agg_sb[:, 3 * T:3 * T + 3],
                                in0=agg_sb[:, 3 * T:3 * T + 3],
                                in1=selp[:], op=OP.add)
            nc.vector.tensor_tensor(out=x_sb[:], in0=x_sb[:], in1=agg_sb[:],
                                    op=OP.add)
            nc.sync.dma_start(
                out=xouts[l - 1][:].rearrange("(T p) f -> p T f", p=128),
                in_=x_sb[:].rearrange("p (T f) -> p T f", f=3))
            if l < 3:
                nc.sync.dma_start(
                    out=xslo[l - 1][:].rearrange("(T p) f -> p T f", p=128),
                    in_=x_sb[:].rearrange("p (T f) -> p T f", f=3))
                nc.gpsimd.collective_compute(
                    "AllGather", OP.bypass,
                    replica_groups=[list(range(n_dev))],
                    ins=[xslo[l - 1][0:npd, :].opt()],
                    outs=[xfulls[l - 1][:].opt()])
    nc.compile()
    return nc


def build_final(batch, N=100000, NG=64):
    """Single-core pooling + classifier. batch: host numpy array (sorted)."""
    nc = bacc.Bacc("TRN2", target_bir_lowering=False, debug=False, num_devices=1)
    xTs = [nc.dram_tensor(f"x{l}T", [3, N], F32, kind="ExternalInput").ap()
           for l in (1, 2, 3)]
    WlT_in = nc.dram_tensor("WlT", [3, 128], F32, kind="ExternalInput").ap()
    WclsT_in = nc.dram_tensor("WclsT", [128, 144], F32, kind="ExternalInput").ap()
    bcls_in = nc.dram_tensor("bcls", [64, 144], F32, kind="ExternalInput").ap()
    out_t = nc.dram_tensor("out", [NG, 144], F32, kind="ExternalOutput").ap()

    bnds = [0]
    for grp in range(NG):
        bnds.append(int(np.searchsorted(batch, grp, side="right")))

    from contextlib import ExitStack
    with tile.TileContext(nc) as tc, ExitStack() as _es:
        cp = _es.enter_context(tc.tile_pool(name="const", bufs=1))
        WlT_t = cp.tile([3, 128], F32)
        nc.sync.dma_start(out=WlT_t[:], in_=WlT_in[:])
        WclsT_t = cp.tile([128, 144], F32)
        nc.sync.dma_start(out=WclsT_t[:], in_=WclsT_in[:])
        bcls_t = cp.tile([64, 144], F32)
        nc.sync.dma_start(out=bcls_t[:], in_=bcls_in[:])
        PSsum = cp.tile([128, NG], F32)
        nc.vector.memset(PSsum[:], 0.0)
        sb = _es.enter_context(tc.tile_pool(name="work", bufs=8))
        ps = _es.enter_context(tc.tile_pool(name="psum", bufs=2, space="PSUM"))
        CH = 2048
        for l in (1, 2, 3):
            Pl = sb.tile([128, NG], F32, tag="Pl")
            nc.vector.memset(Pl[:], -3.0e38)
            for c0 in range(0, N, CH):
                cw = min(CH, N - c0)
                xt = sb.tile([3, CH], F32, tag="xt")
                nc.sync.dma_start(out=xt[:, 0:cw], in_=xTs[l - 1][:, c0:c0 + cw])
                for q0 in range(0, cw, 512):
                    qw = min(512, cw - q0)
                    yp = ps.tile([128, 512], F32, tag="yp")
                    nc.tensor.matmul(out=yp[:, 0:qw], lhsT=WlT_t[:],
                                     rhs=xt[:, q0:q0 + qw], start=True, stop=True)
                    a = c0 + q0
                    bz = a + qw
                    for grp in range(NG):
                        g0 = max(bnds[grp], a)
                        g1 = min(bnds[grp + 1], bz)
                        if g0 >= g1:
                            continue
                        tmp = sb.tile([128, 1], F32, tag="tmp")
                        nc.vector.tensor_reduce(
                            out=tmp[:], in_=yp[:, g0 - a:g1 - a],
                            axis=mybir.AxisListType.X, op=OP.max)
                        nc.vector.tensor_tensor(out=Pl[:, grp:grp + 1],
                                                in0=Pl[:, grp:grp + 1],
                                                in1=tmp[:], op=OP.max)
            nc.vector.tensor_tensor(out=PSsum[:], in0=PSsum[:], in1=Pl[:],
                                    op=OP.add)
        op_ = ps.tile([64, 144], F32, tag="yp")
        nc.tensor.matmul(out=op_[:], lhsT=PSsum[:, 0:64], rhs=WclsT_t[:],
                         start=True, stop=True)
        ot = sb.tile([64, 144], F32, tag="ot")
        nc.vector.tensor_tensor(out=ot[:], in0=op_[:], in1=bcls_t[:], op=OP.add)
        nc.sync.dma_start(out=out_t[:], in_=ot[:])
    nc.compile()
    return nc


_CACHE = {}


def make_in_maps(inputs, devs, shared, n_dev=8):
    x = np.asarray(inputs["x"], np.float32)
    WT = shared["WT"]
    in_maps = []
    for d in range(n_dev):
        dv = devs[d]
        m = dict(x=x, xsl=dv["xsl"], gidx=dv["gidx"], attr_sw=dv["attr_sw"],
                 mrel=dv["mrel"], z1=dv["z1"], S=shared["S"],
                 iota16=shared["iota16"].astype(np.float32))
        for l in (1, 2, 3):
            m[f"Wxd{l}"] = WT[l]["Wxd"]
            m[f"Wa{l}"] = WT[l]["Wa"]
            m[f"bias_g{l}"] = WT[l]["bias_g"]
            m[f"bias_c{l}"] = WT[l]["bias_c"]
            m[f"bias_cm30{l}"] = WT[l]["bias_cm30"]
        in_maps.append(m)
    return in_maps


def kernel(**inputs):
    from concourse import bass_utils
    x = np.asarray(inputs["x"], np.float32)
    ei = np.asarray(inputs["edge_index"])
    ea = np.asarray(inputs["edge_attr"], np.float32)
    batch = np.asarray(inputs["batch"])
    n_dev = 8
    N = x.shape[0]

    devs, shared = preprocess(x, ei, ea, batch, inputs, n_dev=n_dev)
    npd, ntiles, nsb = shared["npd"], shared["ntiles"], shared["nsb"]

    key = ("spmd2", nsb, shared["D8"], ntiles)
    if key not in _CACHE:
        _CACHE[key] = build_spmd(shared, n_dev=n_dev, N=N)
    nc1 = _CACHE[key]

    in_maps = make_in_maps(inputs, devs, shared, n_dev)
    r1 = bass_utils.run_bass_kernel_spmd(nc1, in_maps, core_ids=list(range(n_dev)))

    xTs = {}
    for l in (1, 2, 3):
        full = np.concatenate(
            [r1.results[d][f"xo{l}"][:npd] for d in range(n_dev)], axis=0)
        xTs[l] = np.ascontiguousarray(full.T)

    key2 = ("final", N)
    if key2 not in _CACHE:
        _CACHE[key2] = build_final(np.asarray(batch, np.int64), N=N)
    nc2 = _CACHE[key2]

    W_cls = np.asarray(inputs["W_cls"], np.float32)
    b_eff = (np.asarray(inputs["b_cls"], np.float32)
             + 3.0 * W_cls @ np.asarray(inputs["b_lin"], np.float32))
    fin = dict(x1T=xTs[1], x2T=xTs[2], x3T=xTs[3],
               WlT=np.ascontiguousarray(np.asarray(inputs["W_lin"], np.float32).T),
               WclsT=np.ascontiguousarray(W_cls.T),
               bcls=np.tile(b_eff[None, :], (64, 1)))
    r2 = bass_utils.run_bass_kernel_spmd(nc2, [fin], core_ids=[0])
    return r2.results[0]["out"].astype(np.float32)

